# revision 21
# baseline (speedup 1.0000x reference)
"""Trainium2 Bass kernel for nn_Attention_30227979829300.

Multi-head attention (b=4, n=2048, dim=1024, 16 heads x 64) with
interleaved-pair RoPE + Fourier positional encoding, sharded
tensor-parallel by heads across 8 NeuronCores (2 heads per core).

Per-core plan (all layouts transposed so softmax needs no on-chip
transposes and no max-subtraction):
  - qkv projection: q^T/k^T (and rotate_half variants via row-permuted
    weight copies) / v^T in [head_dim, tokens] layout, f32r matmuls
  - RoPE: q_rope = q*cos + rot(q)*sin + fenc as DVE elementwise ops
  - scores s^T[j, i] = sum_d k[j,d] q[i,d]  (j on partitions)
  - p = exp(s/8) on ACT straight out of PSUM (softmax denominator
    deferred; no max subtraction needed at these magnitudes)
  - out^T[d, i] = sum_j v[j, d] p[j, i], with a fused ones column in the
    stationary operand producing the denominator row for free
  - normalize via fast-reciprocal + GpSimd partition broadcast
  - out-projection with out^T chunks stationary -> token-major partial
    [tokens, 1024] written to DRAM
Host sums the 8 partials (the tensor-parallel all-reduce) and adds b_out.
"""

import sys

if "/opt/trn_rl_repo" not in sys.path:
    sys.path.insert(0, "/opt/trn_rl_repo")

import numpy as np

import concourse.bass as bass
import concourse.tile as tile
from concourse import bacc, mybir
from concourse.bass_utils import run_bass_kernel_spmd

F32 = mybir.dt.float32
F32R = mybir.dt.float32r
ACT_EXP = mybir.ActivationFunctionType.Exp

B, N, DIM = 4, 2048, 1024
HEADS, DH = 16, 64
INNER = HEADS * DH
NF = 16  # fourier freqs
T = B * N  # 8192 flat tokens
NCORES = 8
SCALE = DH ** -0.5


def _build_program():
    nc = bacc.Bacc("TRN2", target_bir_lowering=False, debug=False,
                   num_devices=NCORES)

    d = lambda name, shape, dt, kind: nc.dram_tensor(name, shape, dt, kind=kind).ap()
    xT = d("xT", [DIM, T], F32R, "ExternalInput")
    wq = d("wq", [DIM, 128], F32R, "ExternalInput")
    wqr = d("wqr", [DIM, 128], F32R, "ExternalInput")
    wk = d("wk", [DIM, 128], F32R, "ExternalInput")
    wkr = d("wkr", [DIM, 128], F32R, "ExternalInput")
    wv = d("wv", [DIM, 128], F32R, "ExternalInput")
    wo = d("wo", [128, DIM], F32R, "ExternalInput")
    cos2 = d("cos2", [128, N], F32, "ExternalInput")
    sin2 = d("sin2", [128, N], F32, "ExternalInput")
    fourT = d("fourT", [2 * NF, N], F32R, "ExternalInput")
    wfT = d("wfT", [2 * NF, DH], F32R, "ExternalInput")
    bf = d("bf", [DH, 1], F32, "ExternalInput")
    ident = d("ident", [128, 128], F32, "ExternalInput")
    onesv = d("onesv", [128, 32], F32R, "ExternalInput")
    out = d("out", [T, DIM], F32, "ExternalOutput")

    with tile.TileContext(nc) as tc:
        with tc.tile_pool(name="consts", bufs=1) as consts, \
             tc.tile_pool(name="xt", bufs=16) as xtp, \
             tc.tile_pool(name="qk", bufs=2) as qkp, \
             tc.tile_pool(name="vsb", bufs=2) as vsbp, \
             tc.tile_pool(name="vtmp", bufs=2) as vtmpp, \
             tc.tile_pool(name="ptil", bufs=4) as ptilp, \
             tc.tile_pool(name="ropet", bufs=2) as ropetp, \
             tc.tile_pool(name="outT", bufs=6) as outTp, \
             tc.tile_pool(name="ostg", bufs=4) as ostgp, \
             tc.tile_pool(name="arow", bufs=2) as arowp, \
             tc.tile_pool(name="acc", bufs=2, space="PSUM") as accp, \
             tc.tile_pool(name="pacc", bufs=2, space="PSUM") as paccp, \
             tc.tile_pool(name="avacc", bufs=1, space="PSUM") as avaccp, \
             tc.tile_pool(name="small", bufs=1, space="PSUM") as smallp:

            # ---- load constants ----
            w_sb = {}
            for name, ap in (("wq", wq), ("wqr", wqr), ("wk", wk),
                             ("wkr", wkr), ("wv", wv)):
                t = consts.tile([128, 8 * 128], F32R, tag=name)
                nc.sync.dma_start(
                    t[:].rearrange("p (c d) -> p c d", c=8),
                    ap.rearrange("(c p) d -> p c d", p=128))
                w_sb[name] = t
            id_sb = consts.tile([128, 128], F32, tag="ident")
            nc.sync.dma_start(id_sb[:], ident[:])
            onesv_sb = consts.tile([128, 32], F32R, tag="onesv")
            nc.sync.dma_start(onesv_sb[:], onesv[:])

            # prefetch the first projection block's xT tiles ahead of the
            # big constant DMAs so PE can start as early as possible
            xts0 = []
            for fc in range(8):
                xt_t = xtp.tile([128, 512], F32R, tag="xt")
                nc.sync.dma_start(xt_t[:], xT[bass.ts(fc, 128), 0:512])
                xts0.append(xt_t)

            wo_sb = consts.tile([128, DIM], F32R, tag="wo")
            nc.sync.dma_start(wo_sb[:], wo[:])
            cos_sb = consts.tile([128, N], F32, tag="cos")
            nc.sync.dma_start(cos_sb[:], cos2[:])
            sin_sb = consts.tile([128, N], F32, tag="sin")
            nc.sync.dma_start(sin_sb[:], sin2[:])
            four_sb = consts.tile([2 * NF, N], F32R, tag="four")
            nc.sync.dma_start(four_sb[:], fourT[:])
            wf_sb = consts.tile([2 * NF, DH], F32R, tag="wf")
            nc.sync.dma_start(wf_sb[:], wfT[:])
            bf_sb = consts.tile([DH, 1], F32, tag="bf")
            nc.sync.dma_start(bf_sb[:], bf[:])

            # ---- fenc2 [128, 2048]: fourier @ w_fproj.T + b_fproj, duplicated per head ----
            fenc_sb = consts.tile([128, N], F32, tag="fenc")
            for blk in range(4):
                fp = smallp.tile([DH, 512], F32, tag="small")
                nc.tensor.matmul(fp[:], wf_sb[:], four_sb[:, bass.ts(blk, 512)],
                                 start=True, stop=True)
                nc.scalar.add(fenc_sb[0:64, bass.ts(blk, 512)], fp[:], bf_sb[:])
                nc.scalar.add(fenc_sb[64:128, bass.ts(blk, 512)], fp[:], bf_sb[:])

            UNITS = ("wq", "wqr", "wk", "wkr", "wv")
            batch_tiles = {}

            def proj_block(b, blk):
                """Project q/qr/k/kr/v for 512 tokens of batch b, apply RoPE,
                transpose v to natural layout."""
                if blk == 0:
                    q_rope = qkp.tile([128, N], F32R, tag="q")
                    k_rope = qkp.tile([128, N], F32R, tag="k")
                    v_sb = vsbp.tile([128, 16 * 130], F32R, tag="v")
                    # ones columns (col 64 of each [65]-block, both heads)
                    nc.vector.tensor_copy(
                        bass.AP(tensor=v_sb[:].tensor, offset=v_sb[:].offset + 64,
                                ap=[v_sb[:].ap[0], [130, 16], [65, 2]]),
                        onesv_sb[:].rearrange("p (a t) -> p a t", t=2))
                    batch_tiles[b] = (q_rope, k_rope, v_sb)
                q_rope, k_rope, v_sb = batch_tiles[b]
                tok0 = b * N
                if b == 0 and blk == 0:
                    xts = xts0
                else:
                    xts = []
                    for fc in range(8):
                        xt_t = xtp.tile([128, 512], F32R, tag="xt")
                        nc.sync.dma_start(
                            xt_t[:],
                            xT[bass.ts(fc, 128), tok0 + blk * 512:tok0 + (blk + 1) * 512])
                        xts.append(xt_t)
                pu = {}
                for u in UNITS:
                    p = paccp.tile([128, 512], F32, tag="pacc")
                    for fc in range(8):
                        nc.tensor.matmul(p[:], w_sb[u][:, bass.ts(fc, 128)],
                                         xts[fc][:],
                                         start=(fc == 0), stop=(fc == 7))
                    pu[u] = p
                bsl = bass.ts(blk, 512)
                for base, rotu, dst in (("wq", "wqr", q_rope), ("wk", "wkr", k_rope)):
                    t1 = ropetp.tile([128, 512], F32, tag="t1")
                    nc.vector.tensor_mul(t1[:], pu[base][:], cos_sb[:, bsl])
                    t2 = ropetp.tile([128, 512], F32, tag="t2")
                    nc.vector.tensor_mul(t2[:], pu[rotu][:], sin_sb[:, bsl])
                    t3 = ropetp.tile([128, 512], F32, tag="t3")
                    nc.vector.tensor_add(t3[:], t1[:], t2[:])
                    nc.vector.tensor_add(dst[:, bsl], t3[:], fenc_sb[:, bsl])
                vt = vtmpp.tile([128, 512], F32, tag="vt")
                nc.vector.tensor_copy(vt[:], pu["wv"][:])
                for tt in range(4):
                    jc = blk * 4 + tt
                    ptp = smallp.tile([128, 128], F32, tag="small")
                    nc.tensor.transpose(ptp[:], vt[:, bass.ts(tt, 128)], id_sb[:])
                    nc.vector.tensor_copy(
                        v_sb[:, jc * 130:jc * 130 + 64], ptp[:, 0:64])
                    nc.vector.tensor_copy(
                        v_sb[:, jc * 130 + 65:jc * 130 + 129], ptp[:, 64:128])

            def attn_unit(b, ib, h, ot):
                q_rope, k_rope, v_sb = batch_tiles[b]
                hp = slice(h * 64, (h + 1) * 64)
                q_mv = q_rope[hp, bass.ts(ib, 512)]
                op_ = avaccp.tile([65, 512], F32, tag="av")
                for g in range(8):
                    sg = accp.tile([128, 1024], F32, tag="acc")
                    for t2_ in range(2):
                        jc = g * 2 + t2_
                        nc.tensor.matmul(
                            sg[:, bass.ts(t2_, 512)],
                            k_rope[hp, bass.ts(jc, 128)], q_mv,
                            start=True, stop=True)
                    pt = ptilp.tile([128, 1024], F32R, tag="pt")
                    nc.scalar.activation(pt[:], sg[:], ACT_EXP, scale=SCALE)
                    for t2_ in range(2):
                        jc = g * 2 + t2_
                        nc.tensor.matmul(
                            op_[:],
                            v_sb[:, jc * 130 + h * 65:jc * 130 + h * 65 + 65],
                            pt[:, bass.ts(t2_, 512)],
                            start=(jc == 0), stop=(jc == 15))
                ar0 = arowp.tile([1, 512], F32, tag="ar0")
                nc.vector.tensor_copy(ar0[:], op_[64:65, :])
                ar = arowp.tile([1, 512], F32, tag="ar")
                nc.vector.reciprocal_approx_fast(ar[:], ar0[:])
                bc = arowp.tile([64, 512], F32, tag="bc")
                nc.gpsimd.partition_broadcast(bc[:], ar[:])
                nc.vector.tensor_mul(ot[hp, :], op_[0:64, :], bc[:])

            def outproj(b, ib, ot):
                tok0 = b * N
                for ic in range(4):
                    for oc in range(2):
                        po = smallp.tile([128, 512], F32, tag="small")
                        nc.tensor.matmul(po[:], ot[:, bass.ts(ic, 128)],
                                         wo_sb[:, bass.ts(oc, 512)],
                                         start=True, stop=True)
                        og = ostgp.tile([128, 512], F32, tag="og")
                        nc.vector.tensor_copy(og[:], po[:])
                        r0 = tok0 + ib * 512 + ic * 128
                        nc.sync.dma_start(
                            out[r0:r0 + 128, bass.ts(oc, 512)], og[:])

            for blk in range(4):
                proj_block(0, blk)
            # Out-projections of batches 1 and 2 are deferred by one batch so
            # every ACT-bound attention stretch carries extra PE work (keeps
            # the PE dense enough to hold the HAM clock at full rate).
            deferred = []
            for b in range(B):
                for ib in range(4):
                    ot = outTp.tile([128, 512], F32R, tag="ot")
                    attn_unit(b, ib, 0, ot)
                    attn_unit(b, ib, 1, ot)
                    if b in (1, 2):
                        deferred.append((b, ib, ot))
                    else:
                        outproj(b, ib, ot)
                    if b + 1 < B:
                        proj_block(b + 1, ib)
                    if b >= 2 and deferred:
                        outproj(*deferred.pop(0))

    nc.compile()
    return nc


_NC = None


def _get_nc():
    global _NC
    if _NC is None:
        _NC = _build_program()
    return _NC


def _host_prep(x, w_qkv, w_fproj, b_fproj, w_out, b_out):
    xT = np.ascontiguousarray(x.reshape(T, DIM).T, dtype=np.float32)

    pos = np.arange(N, dtype=np.float64)[:, None]
    freqs = 10000.0 ** (-np.arange(0, DH, 2, dtype=np.float64) / DH)
    ang = pos * freqs
    sin = np.repeat(np.sin(ang), 2, axis=1)  # [N, 64] interleave-dup
    cos = np.repeat(np.cos(ang), 2, axis=1)
    cos2 = np.ascontiguousarray(np.tile(cos.T, (2, 1)), dtype=np.float32)
    sin2 = np.ascontiguousarray(np.tile(sin.T, (2, 1)), dtype=np.float32)
    ff = np.arange(1, NF + 1, dtype=np.float64)
    fourier = np.concatenate([np.sin(pos * ff), np.cos(pos * ff)], axis=1)
    fourT = np.ascontiguousarray(fourier.T, dtype=np.float32)
    wfT = np.ascontiguousarray(w_fproj.T, dtype=np.float32)
    bf = np.ascontiguousarray(b_fproj[:, None], dtype=np.float32)
    identm = np.eye(128, dtype=np.float32)
    onesv = np.ones((128, 32), dtype=np.float32)

    perm = np.empty(DH, np.int64)
    sign = np.empty(DH, np.float32)
    perm[:32] = 2 * np.arange(32) + 1
    sign[:32] = -1.0
    perm[32:] = 2 * np.arange(32)
    sign[32:] = 1.0

    in_maps = []
    for c in range(NCORES):
        rows = np.concatenate([np.arange(h * DH, (h + 1) * DH)
                               for h in (2 * c, 2 * c + 1)])
        Wq = w_qkv[rows]
        Wk = w_qkv[INNER + rows]
        Wv = w_qkv[2 * INNER + rows]

        def rot_w(W):
            Wr = np.empty_like(W)
            for hi in range(2):
                blk = W[hi * 64:(hi + 1) * 64]
                Wr[hi * 64:(hi + 1) * 64] = sign[:, None] * blk[perm]
            return Wr

        ct = lambda a: np.ascontiguousarray(a, dtype=np.float32)
        in_maps.append({
            "xT": xT,
            "wq": ct(Wq.T), "wqr": ct(rot_w(Wq).T),
            "wk": ct(Wk.T), "wkr": ct(rot_w(Wk).T),
            "wv": ct(Wv.T),
            "wo": ct(w_out[:, rows].T),
            "cos2": cos2, "sin2": sin2,
            "fourT": fourT, "wfT": wfT, "bf": bf,
            "ident": identm, "onesv": onesv,
        })
    return in_maps


LAST_RESULT = None


def kernel(x, w_qkv, w_fproj, b_fproj, w_out, b_out, *, trace=False):
    global LAST_RESULT
    x = np.asarray(x, dtype=np.float32)
    w_qkv = np.asarray(w_qkv, dtype=np.float32)
    w_fproj = np.asarray(w_fproj, dtype=np.float32)
    b_fproj = np.asarray(b_fproj, dtype=np.float32)
    w_out = np.asarray(w_out, dtype=np.float32)
    b_out = np.asarray(b_out, dtype=np.float32)

    nc = _get_nc()
    in_maps = _host_prep(x, w_qkv, w_fproj, b_fproj, w_out, b_out)
    res = run_bass_kernel_spmd(nc, in_maps, core_ids=list(range(NCORES)),
                               trace=trace)
    LAST_RESULT = res
    acc = np.zeros((T, DIM), dtype=np.float64)
    for c in range(NCORES):
        acc += res.results[c]["out"]
    acc += b_out
    return acc.reshape(B, N, DIM).astype(np.float32)


# revision 23
# speedup vs baseline: 1.0075x; 1.0075x over previous
"""Trainium2 Bass kernel for nn_Attention_30227979829300.

Multi-head attention (b=4, n=2048, dim=1024, 16 heads x 64) with
interleaved-pair RoPE + Fourier positional encoding, sharded
tensor-parallel by heads across 8 NeuronCores (2 heads per core).

Per-core plan (all layouts transposed so softmax needs no on-chip
transposes and no max-subtraction):
  - qkv projection: q^T/k^T (and rotate_half variants via row-permuted
    weight copies) / v^T in [head_dim, tokens] layout, f32r matmuls
  - RoPE: q_rope = q*cos + rot(q)*sin + fenc as DVE elementwise ops
  - scores s^T[j, i] = sum_d k[j,d] q[i,d]  (j on partitions)
  - p = exp(s/8) on ACT straight out of PSUM (softmax denominator
    deferred; no max subtraction needed at these magnitudes)
  - out^T[d, i] = sum_j v[j, d] p[j, i], with a fused ones column in the
    stationary operand producing the denominator row for free
  - normalize via fast-reciprocal + GpSimd partition broadcast
  - out-projection with out^T chunks stationary -> token-major partial
    [tokens, 1024] written to DRAM
Host sums the 8 partials (the tensor-parallel all-reduce) and adds b_out.
"""

import sys

if "/opt/trn_rl_repo" not in sys.path:
    sys.path.insert(0, "/opt/trn_rl_repo")

import numpy as np

import concourse.bass as bass
import concourse.tile as tile
from concourse import bacc, mybir
from concourse.bass_utils import run_bass_kernel_spmd

F32 = mybir.dt.float32
F32R = mybir.dt.float32r
ACT_EXP = mybir.ActivationFunctionType.Exp

B, N, DIM = 4, 2048, 1024
HEADS, DH = 16, 64
INNER = HEADS * DH
NF = 16  # fourier freqs
T = B * N  # 8192 flat tokens
NCORES = 8
SCALE = DH ** -0.5


def _build_program():
    nc = bacc.Bacc("TRN2", target_bir_lowering=False, debug=False,
                   num_devices=NCORES)

    d = lambda name, shape, dt, kind: nc.dram_tensor(name, shape, dt, kind=kind).ap()
    xT = d("xT", [DIM, T], F32R, "ExternalInput")
    wq = d("wq", [DIM, 128], F32R, "ExternalInput")
    wqr = d("wqr", [DIM, 128], F32R, "ExternalInput")
    wk = d("wk", [DIM, 128], F32R, "ExternalInput")
    wkr = d("wkr", [DIM, 128], F32R, "ExternalInput")
    wv = d("wv", [DIM, 128], F32R, "ExternalInput")
    wo = d("wo", [128, DIM], F32R, "ExternalInput")
    cos2 = d("cos2", [128, N], F32, "ExternalInput")
    sin2 = d("sin2", [128, N], F32, "ExternalInput")
    fourT = d("fourT", [2 * NF, N], F32R, "ExternalInput")
    wfT = d("wfT", [2 * NF, DH], F32R, "ExternalInput")
    bf = d("bf", [DH, 1], F32, "ExternalInput")
    ident = d("ident", [128, 128], F32, "ExternalInput")
    onesv = d("onesv", [128, 32], F32R, "ExternalInput")
    out = d("out", [T, DIM], F32, "ExternalOutput")

    with tile.TileContext(nc) as tc:
        with tc.tile_pool(name="consts", bufs=1) as consts, \
             tc.tile_pool(name="xt", bufs=16) as xtp, \
             tc.tile_pool(name="qk", bufs=2) as qkp, \
             tc.tile_pool(name="vsb", bufs=2) as vsbp, \
             tc.tile_pool(name="vtmp", bufs=2) as vtmpp, \
             tc.tile_pool(name="ptil", bufs=4) as ptilp, \
             tc.tile_pool(name="ropet", bufs=2) as ropetp, \
             tc.tile_pool(name="outT", bufs=6) as outTp, \
             tc.tile_pool(name="ostg", bufs=4) as ostgp, \
             tc.tile_pool(name="arow", bufs=2) as arowp, \
             tc.tile_pool(name="acc", bufs=2, space="PSUM") as accp, \
             tc.tile_pool(name="pacc", bufs=2, space="PSUM") as paccp, \
             tc.tile_pool(name="avacc", bufs=1, space="PSUM") as avaccp, \
             tc.tile_pool(name="small", bufs=1, space="PSUM") as smallp:

            # ---- load constants ----
            w_sb = {}
            for name, ap in (("wq", wq), ("wqr", wqr), ("wk", wk),
                             ("wkr", wkr), ("wv", wv)):
                t = consts.tile([128, 8 * 128], F32R, tag=name)
                nc.sync.dma_start(
                    t[:].rearrange("p (c d) -> p c d", c=8),
                    ap.rearrange("(c p) d -> p c d", p=128))
                w_sb[name] = t
            id_sb = consts.tile([128, 128], F32, tag="ident")
            nc.sync.dma_start(id_sb[:], ident[:])
            onesv_sb = consts.tile([128, 32], F32R, tag="onesv")
            nc.sync.dma_start(onesv_sb[:], onesv[:])

            # prefetch the first projection block's xT tiles ahead of the
            # big constant DMAs so PE can start as early as possible
            xts0 = []
            for fc in range(8):
                xt_t = xtp.tile([128, 512], F32R, tag="xt")
                nc.sync.dma_start(xt_t[:], xT[bass.ts(fc, 128), 0:512])
                xts0.append(xt_t)

            wo_sb = consts.tile([128, DIM], F32R, tag="wo")
            nc.sync.dma_start(wo_sb[:], wo[:])
            cos_sb = consts.tile([128, N], F32, tag="cos")
            nc.sync.dma_start(cos_sb[:], cos2[:])
            sin_sb = consts.tile([128, N], F32, tag="sin")
            nc.sync.dma_start(sin_sb[:], sin2[:])
            four_sb = consts.tile([2 * NF, N], F32R, tag="four")
            nc.sync.dma_start(four_sb[:], fourT[:])
            wf_sb = consts.tile([2 * NF, DH], F32R, tag="wf")
            nc.sync.dma_start(wf_sb[:], wfT[:])
            bf_sb = consts.tile([DH, 1], F32, tag="bf")
            nc.sync.dma_start(bf_sb[:], bf[:])

            # ---- fenc2 [128, 2048]: fourier @ w_fproj.T + b_fproj, duplicated per head ----
            fenc_sb = consts.tile([128, N], F32, tag="fenc")
            for blk in range(4):
                fp = smallp.tile([DH, 512], F32, tag="small")
                nc.tensor.matmul(fp[:], wf_sb[:], four_sb[:, bass.ts(blk, 512)],
                                 start=True, stop=True)
                nc.scalar.add(fenc_sb[0:64, bass.ts(blk, 512)], fp[:], bf_sb[:])
                nc.scalar.add(fenc_sb[64:128, bass.ts(blk, 512)], fp[:], bf_sb[:])

            UNITS = ("wq", "wqr", "wk", "wkr", "wv")
            batch_tiles = {}

            def proj_block(b, blk):
                """Project q/qr/k/kr/v for 512 tokens of batch b, apply RoPE,
                transpose v to natural layout."""
                if blk == 0:
                    q_rope = qkp.tile([128, N], F32R, tag="q")
                    k_rope = qkp.tile([128, N], F32R, tag="k")
                    v_sb = vsbp.tile([128, 16 * 130], F32R, tag="v")
                    # ones columns (col 64 of each [65]-block, both heads)
                    nc.vector.tensor_copy(
                        bass.AP(tensor=v_sb[:].tensor, offset=v_sb[:].offset + 64,
                                ap=[v_sb[:].ap[0], [130, 16], [65, 2]]),
                        onesv_sb[:].rearrange("p (a t) -> p a t", t=2))
                    batch_tiles[b] = (q_rope, k_rope, v_sb)
                q_rope, k_rope, v_sb = batch_tiles[b]
                tok0 = b * N
                if b == 0 and blk == 0:
                    xts = xts0
                else:
                    xts = []
                    for fc in range(8):
                        xt_t = xtp.tile([128, 512], F32R, tag="xt")
                        nc.sync.dma_start(
                            xt_t[:],
                            xT[bass.ts(fc, 128), tok0 + blk * 512:tok0 + (blk + 1) * 512])
                        xts.append(xt_t)
                pu = {}
                for u in UNITS:
                    p = paccp.tile([128, 512], F32, tag="pacc")
                    for fc in range(8):
                        nc.tensor.matmul(p[:], w_sb[u][:, bass.ts(fc, 128)],
                                         xts[fc][:],
                                         start=(fc == 0), stop=(fc == 7))
                    pu[u] = p
                bsl = bass.ts(blk, 512)
                for base, rotu, dst in (("wq", "wqr", q_rope), ("wk", "wkr", k_rope)):
                    t1 = ropetp.tile([128, 512], F32, tag="t1")
                    nc.vector.tensor_mul(t1[:], pu[base][:], cos_sb[:, bsl])
                    t2 = ropetp.tile([128, 512], F32, tag="t2")
                    nc.vector.tensor_mul(t2[:], pu[rotu][:], sin_sb[:, bsl])
                    t3 = ropetp.tile([128, 512], F32, tag="t3")
                    nc.vector.tensor_add(t3[:], t1[:], t2[:])
                    nc.vector.tensor_add(dst[:, bsl], t3[:], fenc_sb[:, bsl])
                vt = vtmpp.tile([128, 512], F32, tag="vt")
                nc.vector.tensor_copy(vt[:], pu["wv"][:])
                for tt in range(4):
                    jc = blk * 4 + tt
                    ptp = smallp.tile([128, 128], F32, tag="small")
                    nc.tensor.transpose(ptp[:], vt[:, bass.ts(tt, 128)], id_sb[:])
                    nc.vector.tensor_copy(
                        v_sb[:, jc * 130:jc * 130 + 64], ptp[:, 0:64])
                    nc.vector.tensor_copy(
                        v_sb[:, jc * 130 + 65:jc * 130 + 129], ptp[:, 64:128])

            def attn_unit(b, ib, h, ot):
                q_rope, k_rope, v_sb = batch_tiles[b]
                hp = slice(h * 64, (h + 1) * 64)
                q_mv = q_rope[hp, bass.ts(ib, 512)]
                op_ = avaccp.tile([65, 512], F32, tag="av")
                for g in range(8):
                    sg = accp.tile([128, 1024], F32, tag="acc")
                    for t2_ in range(2):
                        jc = g * 2 + t2_
                        nc.tensor.matmul(
                            sg[:, bass.ts(t2_, 512)],
                            k_rope[hp, bass.ts(jc, 128)], q_mv,
                            start=True, stop=True)
                    pt = ptilp.tile([128, 1024], F32R, tag="pt")
                    nc.scalar.activation(pt[:], sg[:], ACT_EXP, scale=SCALE)
                    for t2_ in range(2):
                        jc = g * 2 + t2_
                        nc.tensor.matmul(
                            op_[:],
                            v_sb[:, jc * 130 + h * 65:jc * 130 + h * 65 + 65],
                            pt[:, bass.ts(t2_, 512)],
                            start=(jc == 0), stop=(jc == 15))
                ar0 = arowp.tile([1, 512], F32, tag="ar0")
                nc.vector.tensor_copy(ar0[:], op_[64:65, :])
                ar = arowp.tile([1, 512], F32, tag="ar")
                nc.vector.reciprocal_approx_fast(ar[:], ar0[:])
                bc = arowp.tile([64, 512], F32, tag="bc")
                nc.gpsimd.partition_broadcast(bc[:], ar[:])
                nc.vector.tensor_mul(ot[hp, :], op_[0:64, :], bc[:])

            def outproj(b, ib, ot):
                tok0 = b * N
                for ic in range(4):
                    for oc in range(2):
                        po = smallp.tile([128, 512], F32, tag="small")
                        nc.tensor.matmul(po[:], ot[:, bass.ts(ic, 128)],
                                         wo_sb[:, bass.ts(oc, 512)],
                                         start=True, stop=True)
                        og = ostgp.tile([128, 512], F32, tag="og")
                        nc.vector.tensor_copy(og[:], po[:])
                        r0 = tok0 + ib * 512 + ic * 128
                        nc.sync.dma_start(
                            out[r0:r0 + 128, bass.ts(oc, 512)], og[:])

            for blk in range(4):
                proj_block(0, blk)
            # Out-projections of batches 1 and 2 are deferred by one batch so
            # every ACT-bound attention stretch carries extra PE work (keeps
            # the PE dense enough to hold the HAM clock at full rate).
            deferred = []
            for b in range(B):
                for ib in range(4):
                    ot = outTp.tile([128, 512], F32R, tag="ot")
                    attn_unit(b, ib, 0, ot)
                    attn_unit(b, ib, 1, ot)
                    if b in (1, 2):
                        deferred.append((b, ib, ot))
                    else:
                        outproj(b, ib, ot)
                    if b + 1 < B:
                        proj_block(b + 1, ib)
                    if b >= 2 and deferred:
                        outproj(*deferred.pop(0))

    nc.compile()
    return nc


_NC = None


def _get_nc():
    global _NC
    if _NC is None:
        _NC = _build_program()
    return _NC


def _host_prep(x, w_qkv, w_fproj, b_fproj, w_out, b_out):
    xT = np.ascontiguousarray(x.reshape(T, DIM).T, dtype=np.float32)

    pos = np.arange(N, dtype=np.float64)[:, None]
    freqs = 10000.0 ** (-np.arange(0, DH, 2, dtype=np.float64) / DH)
    ang = pos * freqs
    sin = np.repeat(np.sin(ang), 2, axis=1)  # [N, 64] interleave-dup
    cos = np.repeat(np.cos(ang), 2, axis=1)
    cos2 = np.ascontiguousarray(np.tile(cos.T, (2, 1)), dtype=np.float32)
    sin2 = np.ascontiguousarray(np.tile(sin.T, (2, 1)), dtype=np.float32)
    ff = np.arange(1, NF + 1, dtype=np.float64)
    fourier = np.concatenate([np.sin(pos * ff), np.cos(pos * ff)], axis=1)
    fourT = np.ascontiguousarray(fourier.T, dtype=np.float32)
    wfT = np.ascontiguousarray(w_fproj.T, dtype=np.float32)
    bf = np.ascontiguousarray(b_fproj[:, None], dtype=np.float32)
    identm = np.eye(128, dtype=np.float32)
    onesv = np.ones((128, 32), dtype=np.float32)

    perm = np.empty(DH, np.int64)
    sign = np.empty(DH, np.float32)
    perm[:32] = 2 * np.arange(32) + 1
    sign[:32] = -1.0
    perm[32:] = 2 * np.arange(32)
    sign[32:] = 1.0

    in_maps = []
    for c in range(NCORES):
        rows = np.concatenate([np.arange(h * DH, (h + 1) * DH)
                               for h in (2 * c, 2 * c + 1)])
        Wq = w_qkv[rows]
        Wk = w_qkv[INNER + rows]
        Wv = w_qkv[2 * INNER + rows]

        def rot_w(W):
            Wr = np.empty_like(W)
            for hi in range(2):
                blk = W[hi * 64:(hi + 1) * 64]
                Wr[hi * 64:(hi + 1) * 64] = sign[:, None] * blk[perm]
            return Wr

        ct = lambda a: np.ascontiguousarray(a, dtype=np.float32)
        in_maps.append({
            "xT": xT,
            "wq": ct(Wq.T), "wqr": ct(rot_w(Wq).T),
            "wk": ct(Wk.T), "wkr": ct(rot_w(Wk).T),
            "wv": ct(Wv.T),
            "wo": ct(w_out[:, rows].T),
            "cos2": cos2, "sin2": sin2,
            "fourT": fourT, "wfT": wfT, "bf": bf,
            "ident": identm, "onesv": onesv,
        })
    return in_maps


LAST_RESULT = None


def kernel(x, w_qkv, w_fproj, b_fproj, w_out, b_out, *, trace=False):
    global LAST_RESULT
    x = np.asarray(x, dtype=np.float32)
    w_qkv = np.asarray(w_qkv, dtype=np.float32)
    w_fproj = np.asarray(w_fproj, dtype=np.float32)
    b_fproj = np.asarray(b_fproj, dtype=np.float32)
    w_out = np.asarray(w_out, dtype=np.float32)
    b_out = np.asarray(b_out, dtype=np.float32)

    nc = _get_nc()
    in_maps = _host_prep(x, w_qkv, w_fproj, b_fproj, w_out, b_out)
    res = run_bass_kernel_spmd(nc, in_maps, core_ids=list(range(NCORES)),
                               trace=trace)
    LAST_RESULT = res
    acc = np.zeros((T, DIM), dtype=np.float64)
    for c in range(NCORES):
        acc += res.results[c]["out"]
    acc += b_out
    return acc.reshape(B, N, DIM).astype(np.float32)
